# revision 12
# baseline (speedup 1.0000x reference)
"""Trainium2 Bass kernel for nn_Conv1DTokenEncoder.

Math: the reference computes, per (b,t) row of length L=1024,
  out[b,t,d] = (1/L) * sum_k w[d,k] * S[b,t,k] + bias[d]
with S the windowed sums of the zero-padded row. For K=5, pad=2 the S's
collapse to the row total minus edge elements, so with host-precomputed
M6 [6, D]:
  out[r, :] = [total, x0, x1, xL2, xL1, 1] @ M6
where M6 rows are [wsum/L, -(w3+w4)/L, -w4/L, -w0/L, -(w0+w1)/L, bias].
This turns the conv into a pure memory-bound row reduction plus a tiny
K=6 matmul.

The correctness gate is rel_err < 2e-2, so x and out travel in fp16
(measured end-to-end scale_rel ~ 7e-4): HBM traffic drops from
16+8 MiB to 8+4 MiB per core.

Device structure (per core, 4096 rows; measured on HW via ablations):
- "Fat partitions": each SBUF partition holds G=8 consecutive DRAM rows
  so HBM descriptors are 16 KiB contiguous runs; 4 blocks of 1024 rows.
- Input: 2 sync-HWDGE DMAs per block (measured read bw ~650 GB/s/core;
  the sync queue carries only always-ready input loads so its in-order
  queue never head-of-line blocks on compute).
- Row totals: VectorE folds xt [128,8,1024] -> 512 -> 256 -> 128 with
  fp16 tensor_tensor adds (2x 16-bit mode), then one tensor_reduce to
  f32 totals. This is ~2x cheaper than activation-accumulate reduction.
- Features [total, x0, x1, xL2, xL1, 1] assembled in fp16 on ScalarE;
  PE transposes them via identity matmul (fp16, 1 cyc/row); ftT slices
  cast PSUM->SBUF on ScalarE; fp16 matmuls against M6 [6, 512]; paired
  PSUM banks cast f32->fp16 to SBUF (ScalarE, 1 per block on VectorE).
- Output: one batched [128, G, D] DMA per block on the gpsimd SWDGE
  ring (keeps ScalarE free of ~1.3us/issue HWDGE setup cost; writes
  measured ~200 GB/s/core are the envelope's slow half).
Pure data parallel across 8 cores (batch*token rows sharded).
"""

import numpy as np

B, T, L, D = 16, 2048, 1024, 512
N_CORES = 8
BT = B * T
ROWS_PER_CORE = BT // N_CORES  # 4096
P = 128

G = 8                       # rows per partition per block
BLOCK_ROWS = P * G          # 1024
N_BLOCKS = ROWS_PER_CORE // BLOCK_ROWS  # 4
FC = 32                     # feature cols per row-group (6 used + pad)

_CACHE = {}


def _build(repeat: int = 1):
    import concourse.bass as bass
    import concourse.tile as tile
    from concourse import bacc, mybir

    f32 = mybir.dt.float32
    f16 = mybir.dt.float16
    nc = bacc.Bacc("TRN2", target_bir_lowering=False, debug=False)

    x_d = nc.dram_tensor("x", [ROWS_PER_CORE, L], f16, kind="ExternalInput")
    m_d = nc.dram_tensor("m6", [6, D], f16, kind="ExternalInput")
    id_d = nc.dram_tensor("ident", [P, P], f16, kind="ExternalInput")
    o_d = nc.dram_tensor("out", [ROWS_PER_CORE, D], f16, kind="ExternalOutput")

    AF = mybir.ActivationFunctionType
    x_v = x_d.ap().rearrange("(nb p g) l -> nb p g l", p=P, g=G)
    o_v = o_d.ap().rearrange("(nb p g) d -> nb p g d", p=P, g=G)

    with tile.TileContext(nc) as tc:
        with (
            tc.tile_pool(name="const", bufs=1) as constp,
            tc.tile_pool(name="xin", bufs=3) as xin,
            tc.tile_pool(name="fold", bufs=2) as foldp,
            tc.tile_pool(name="tot", bufs=2) as totp,
            tc.tile_pool(name="feat", bufs=2) as featp,
            tc.tile_pool(name="ftT_ps", bufs=2, space="PSUM") as ftp,
            tc.tile_pool(name="ftT_sb", bufs=8) as fts,
            tc.tile_pool(name="out_ps", bufs=3, space="PSUM") as outp,
            tc.tile_pool(name="out_sb", bufs=2) as outs,
        ):
            m6 = constp.tile([6, D], f16)
            nc.sync.dma_start(m6[:], m_d[:])
            ident = constp.tile([P, P], f16)
            nc.sync.dma_start(ident[:], id_d[:])

            def body():
                for i in range(N_BLOCKS):
                    xt = xin.tile([P, G, L], f16)
                    h = G // 2
                    nc.sync.dma_start(xt[:, :h, :], x_v[i, :, :h, :])
                    nc.sync.dma_start(xt[:, h:, :], x_v[i, :, h:, :])

                    # fp16 fold chain on VectorE (2x 16-bit mode)
                    f1 = foldp.tile([P, G, L // 2], f16)
                    nc.vector.tensor_tensor(
                        f1[:], xt[:, :, : L // 2], xt[:, :, L // 2 :],
                        op=mybir.AluOpType.add,
                    )
                    f2 = foldp.tile([P, G, L // 4], f16)
                    nc.vector.tensor_tensor(
                        f2[:], f1[:, :, : L // 4], f1[:, :, L // 4 :],
                        op=mybir.AluOpType.add,
                    )
                    f3 = foldp.tile([P, G, L // 8], f16)
                    nc.vector.tensor_tensor(
                        f3[:], f2[:, :, : L // 8], f2[:, :, L // 8 :],
                        op=mybir.AluOpType.add,
                    )
                    tot = totp.tile([P, G], f32)
                    nc.vector.tensor_reduce(
                        tot[:], f3[:],
                        axis=mybir.AxisListType.X, op=mybir.AluOpType.add,
                    )

                    # feature tile (fp16): [total, x0, x1, xL2, xL1, 1, pad]
                    ft = featp.tile([P, G, FC], f16)
                    # ones column + pad cols via ScalarE: out = in*0 + 1
                    nc.scalar.activation(
                        ft[:, :, 5:], xt[:, :, 5:FC], AF.Copy,
                        bias=1.0, scale=0.0,
                    )
                    nc.scalar.activation(ft[:, :, 0:1], tot[:, :, None], AF.Copy)
                    nc.scalar.activation(ft[:, :, 1:3], xt[:, :, 0:2], AF.Copy)
                    nc.scalar.activation(
                        ft[:, :, 3:5], xt[:, :, L - 2 : L], AF.Copy
                    )

                    fps = []
                    for t in range(2):
                        ftT_p = ftp.tile([P, P], f16)
                        nc.tensor.transpose(
                            ftT_p[:],
                            ft[:, 4 * t : 4 * t + 4, :].rearrange(
                                "p g c -> p (g c)"
                            ),
                            ident[:],
                        )
                        fps.append(ftT_p)

                    ot = outs.tile([P, G, D], f16)
                    ops = []
                    for j in range(G):
                        ftT = fts.tile([6, P], f16)
                        nc.scalar.activation(
                            ftT[:],
                            fps[j // 4][FC * (j % 4) : FC * (j % 4) + 6, :],
                            AF.Copy,
                        )
                        if j % 2 == 0:
                            op = outp.tile([P, 2, D], f32)
                            ops.append(op)
                        op = ops[j // 2]
                        nc.tensor.matmul(op[:, j % 2, :], ftT[:], m6[:])
                        if j % 2 == 1:
                            # paired-bank PSUM->SBUF fp16 casts, split
                            # ScalarE/VectorE to balance engine busy time
                            # (GPSIMD cannot read PSUM)
                            if j in (1, 5):
                                nc.vector.tensor_copy(
                                    ot[:, j - 1 : j + 1, :], op[:]
                                )
                            else:
                                nc.scalar.activation(
                                    ot[:, j - 1 : j + 1, :], op[:], AF.Copy
                                )
                    # batched output DMA on the gpsimd SWDGE ring
                    nc.gpsimd.dma_start(o_v[i], ot[:])

            if repeat == 1:
                body()
            else:
                with tc.For_i(0, repeat, 1):
                    body()

    nc.compile()
    return nc


def _host_m6(w: np.ndarray, b: np.ndarray) -> np.ndarray:
    w = w.astype(np.float64)
    invL = 1.0 / L
    rows = [
        w.sum(axis=1) * invL,            # total
        -(w[:, 3] + w[:, 4]) * invL,     # x[0]
        -w[:, 4] * invL,                 # x[1]
        -w[:, 0] * invL,                 # x[L-2]
        -(w[:, 0] + w[:, 1]) * invL,     # x[L-1]
        b.astype(np.float64),            # ones
    ]
    return np.stack(rows).astype(np.float16)


def kernel(x: np.ndarray, w: np.ndarray, b: np.ndarray) -> np.ndarray:
    from concourse.bass_utils import run_bass_kernel_spmd

    if "nc" not in _CACHE:
        _CACHE["nc"] = _build()
    nc = _CACHE["nc"]

    m6 = _host_m6(w, b)
    ident = np.eye(P, dtype=np.float16)
    shards = np.ascontiguousarray(x.astype(np.float16).reshape(BT, L)).reshape(
        N_CORES, ROWS_PER_CORE, L
    )
    in_maps = [
        {"x": shards[i], "m6": m6, "ident": ident} for i in range(N_CORES)
    ]
    res = run_bass_kernel_spmd(nc, in_maps, list(range(N_CORES))).results
    out = np.concatenate([res[i]["out"] for i in range(N_CORES)], axis=0)
    return out.astype(np.float32).reshape(B, T, D)
